# revision 5
# baseline (speedup 1.0000x reference)
"""Trainium2 Bass kernel for nn_MLP_Interpolate.

Reference computation (out_size=512, H=W=128, so exact 4x nearest upsample):
  out[b, :, 4k+r, 4l+s] = relu(x[b,:,k,l] @ W1[:64] + c[r,s]) @ W2 + b2
  c[r,s] = rel_y(r)*W1[64] + rel_x(s)*W1[65] + b1,  rel(t) = (2t-3)/4

Strategy (8 cores, shard = (batch, H-half)):
  - F = W1c^T x computed on PE with a block-diagonal stationary so two
    64-channel pixel groups share one pass (128 partitions fully used).
  - 16 bias+relu variants split across ACT and DVE, written into an
    interleaved rhs tile ordered by *output* column (4l+s).
  - pred on PE with block-diag [128,6] W2 stationary -> PSUM rows are
    whole contiguous output rows, DMA'd straight to DRAM.
"""

import numpy as np

import concourse.bass as bass
import concourse.bacc as bacc
import concourse.mybir as mybir
import concourse.tile as tile
from concourse.bass_utils import run_bass_kernel_spmd

# Problem constants (hardcoded per contract)
B, C, H, W = 4, 64, 128, 128
OUT = 512
NF = 64  # n_feat
N_CORES = 8
ROWS_PER_CORE = H // 2          # 64 input rows per core
T_TILES = ROWS_PER_CORE // 8    # 8 F-tiles, each covering 8 input rows
REL = np.array([-0.75, -0.25, 0.25, 0.75], dtype=np.float32)

_CACHE = {}


def _build_program():
    """Build + compile the SPMD Bass program once."""
    if "nc" in _CACHE:
        return _CACHE["nc"]

    fp32 = mybir.dt.float32
    nc = bacc.Bacc("TRN2", target_bir_lowering=False, debug=False,
                   num_devices=N_CORES)

    x_d = nc.dram_tensor("x", [C, ROWS_PER_CORE, W], fp32, kind="ExternalInput")
    w1_d = nc.dram_tensor("w1diag", [128, 128], fp32, kind="ExternalInput")
    w2_d = nc.dram_tensor("w2diag", [128, 6], fp32, kind="ExternalInput")
    crs_d = nc.dram_tensor("crsT", [128, 16], fp32, kind="ExternalInput")
    out_d = nc.dram_tensor("out", [3, 4 * ROWS_PER_CORE, OUT], fp32,
                           kind="ExternalOutput")

    with tile.TileContext(nc) as tc:
        with (
            tc.tile_pool(name="consts", bufs=1) as consts,
            tc.tile_pool(name="xin", bufs=3) as xin,
            tc.tile_pool(name="hbuf", bufs=3) as hbuf,
            tc.tile_pool(name="stage", bufs=4) as stage,
            tc.tile_pool(name="fpsum", bufs=2, space=bass.MemorySpace.PSUM) as fpsum,
            tc.tile_pool(name="ppsum", bufs=3, space=bass.MemorySpace.PSUM) as ppsum,
        ):
            w1_sb = consts.tile([128, 128], fp32)
            w2_sb = consts.tile([128, 6], fp32)
            crs_sb = consts.tile([128, 16], fp32)
            nc.sync.dma_start(w1_sb[:], w1_d[:])
            nc.sync.dma_start(w2_sb[:], w2_d[:])
            nc.sync.dma_start(crs_sb[:], crs_d[:])

            x_tiles = []
            f_tiles = []

            def load_x(t):
                xt = xin.tile([128, 4, W], fp32, tag="xt")
                # group A: rows 8t..8t+3 -> partitions 0..63 (64 channels)
                nc.sync.dma_start(xt[0:64, :, :], x_d[:, 8 * t:8 * t + 4, :])
                # group B: rows 8t+4..8t+7 -> partitions 64..127
                nc.sync.dma_start(xt[64:128, :, :], x_d[:, 8 * t + 4:8 * t + 8, :])
                x_tiles.append(xt)

            def feat_matmul(t):
                ft = fpsum.tile([128, 4, W], fp32, tag="ft")
                nc.tensor.matmul(ft[:, :, :], w1_sb[:], x_tiles[t][:, :, :],
                                 start=True, stop=True)
                f_tiles.append(ft)

            # 9 relu variants on ACT, 7 on DVE (balances 1.2 vs 0.96 GHz)
            ACT_V = {0, 2, 4, 6, 8, 10, 12, 14, 15}

            def tile_body(t):
                ft = f_tiles[t]
                for r in range(4):
                    # rhs tile ordered by output column: [part, i, l, s]
                    hr = hbuf.tile([128, 4, W, 4], fp32, tag="hr")
                    for s in range(4):
                        v = 4 * r + s
                        bias_ap = crs_sb[:, v:v + 1]
                        if v in ACT_V:
                            nc.scalar.activation(
                                hr[:, :, :, s], ft[:, :, :],
                                mybir.ActivationFunctionType.Relu,
                                bias=bias_ap)
                        else:
                            nc.vector.tensor_scalar(
                                hr[:, :, :, s], ft[:, :, :],
                                bias_ap, 0.0,
                                mybir.AluOpType.add, mybir.AluOpType.max)
                    # i-blocks at (partition 32*(i//2), slot i%2): matmul
                    # out base partition must be 0/32/64
                    pt = ppsum.tile([38, 2, OUT], fp32, tag="pt")
                    for i in range(4):
                        g, j = 32 * (i // 2), i % 2
                        nc.tensor.matmul(pt[g:g + 6, j, :], w2_sb[:],
                                         hr[:, i, :, :],
                                         start=True, stop=True)
                    st = stage.tile([38, 2, OUT], fp32, tag="st")
                    if r % 2 == 0:
                        nc.scalar.activation(
                            st[:, :, :], pt[:, :, :],
                            mybir.ActivationFunctionType.Copy)
                    else:
                        nc.vector.tensor_copy(st[:, :, :], pt[:, :, :])
                    # partitions g+3grp+c (grp: 0=A rows, 1=B rows+16)
                    for q in range(2):       # q = i//2 partition group
                        for grp in range(2):
                            row = 32 * t + 8 * q + 16 * grp + r
                            nc.sync.dma_start(
                                out_d[:, row:row + 5:4, :],
                                st[32 * q + 3 * grp:32 * q + 3 * grp + 3,
                                   :, :])

            # software pipeline: F(t+1) issued before preds(t) so ACT/DVE
            # for tile t+1 overlap PE pred work of tile t
            load_x(0)
            feat_matmul(0)
            for t in range(T_TILES):
                if t + 1 < T_TILES:
                    load_x(t + 1)
                    feat_matmul(t + 1)
                tile_body(t)

    nc.compile()
    _CACHE["nc"] = nc
    return nc


def _prep_inputs(x, W1, b1, W2, b2):
    x = np.ascontiguousarray(np.asarray(x, dtype=np.float32))
    W1 = np.asarray(W1, dtype=np.float32)
    b1 = np.asarray(b1, dtype=np.float32)
    W2 = np.asarray(W2, dtype=np.float32)

    w1c = W1[:NF]                      # [64, 64]
    w1diag = np.zeros((128, 128), dtype=np.float32)
    w1diag[0:64, 0:64] = w1c
    w1diag[64:128, 64:128] = w1c

    w2diag = np.zeros((128, 6), dtype=np.float32)
    w2diag[0:64, 0:3] = W2
    w2diag[64:128, 3:6] = W2

    # c[r,s] = rel[r]*W1[64] + rel[s]*W1[65] + b1 -> [16, 64]
    crs = (REL[:, None, None] * W1[NF][None, None, :]
           + REL[None, :, None] * W1[NF + 1][None, None, :]
           + b1[None, None, :]).reshape(16, NF)
    crsT = np.ascontiguousarray(
        np.concatenate([crs.T, crs.T], axis=0))  # [128, 16]

    in_maps = []
    for c in range(N_CORES):
        b, half = c // 2, c % 2
        xs = np.ascontiguousarray(
            x[b, :, half * ROWS_PER_CORE:(half + 1) * ROWS_PER_CORE, :])
        in_maps.append({"x": xs, "w1diag": w1diag, "w2diag": w2diag,
                        "crsT": crsT})
    return in_maps


def _gather(results, b2):
    full = np.empty((B, 3, OUT, OUT), dtype=np.float32)
    for c in range(N_CORES):
        b, half = c // 2, c % 2
        full[b, :, half * (OUT // 2):(half + 1) * (OUT // 2), :] = \
            results[c]["out"]
    b2 = np.asarray(b2, dtype=np.float32)
    if np.any(b2):
        full += b2.reshape(1, 3, 1, 1)
    return full


def run(trace=False, **inputs):
    nc = _build_program()
    in_maps = _prep_inputs(inputs["x"], inputs["W1"], inputs["b1"],
                           inputs["W2"], inputs["b2"])
    res = run_bass_kernel_spmd(nc, in_maps, list(range(N_CORES)), trace=trace)
    return _gather(res.results, inputs["b2"]), res


def kernel(**inputs):
    out, _ = run(trace=False, **inputs)
    return out


# revision 11
# speedup vs baseline: 1.0100x; 1.0100x over previous
"""Trainium2 Bass kernel for nn_MLP_Interpolate.

Reference computation (out_size=512, H=W=128, so exact 4x nearest upsample):
  out[b, :, 4k+r, 4l+s] = relu(x[b,:,k,l] @ W1[:64] + c[r,s]) @ W2 + b2
  c[r,s] = rel_y(r)*W1[64] + rel_x(s)*W1[65] + b1,  rel(t) = (2t-3)/4

Strategy (8 cores, shard = (batch, H-half)):
  - F = W1c^T x computed on PE with a block-diagonal stationary so two
    64-channel pixel groups share one pass (128 partitions fully used).
  - 16 bias+relu variants split across ACT and DVE, written into an
    interleaved rhs tile ordered by *output* column (4l+s).
  - pred on PE with block-diag [128,6] W2 stationary -> PSUM rows are
    whole contiguous output rows, DMA'd straight to DRAM.
"""

import os

import numpy as np

import concourse.bass as bass
import concourse.bacc as bacc
import concourse.mybir as mybir
import concourse.tile as tile
from concourse.bass_utils import run_bass_kernel_spmd

# Problem constants (hardcoded per contract)
B, C, H, W = 4, 64, 128, 128
OUT = 512
NF = 64  # n_feat
N_CORES = 8
ROWS_PER_CORE = H // 2          # 64 input rows per core
T_TILES = ROWS_PER_CORE // 8    # 8 F-tiles, each covering 8 input rows
REL = np.array([-0.75, -0.25, 0.25, 0.75], dtype=np.float32)

_CACHE = {}


def _build_program():
    """Build + compile the SPMD Bass program once."""
    if "nc" in _CACHE:
        return _CACHE["nc"]

    fp32 = mybir.dt.float32
    # float32r: same bytes as fp32, PE streams 1 col/cycle vs 4 for fp32
    mm_dt = (mybir.dt.float32r if os.environ.get("MM_DTYPE") == "f32r"
             else fp32)
    nc = bacc.Bacc("TRN2", target_bir_lowering=False, debug=False,
                   num_devices=N_CORES)

    x_d = nc.dram_tensor("x", [C, ROWS_PER_CORE, W], mm_dt, kind="ExternalInput")
    w1_d = nc.dram_tensor("w1diag", [128, 128], mm_dt, kind="ExternalInput")
    w2_d = nc.dram_tensor("w2diag", [128, 6], mm_dt, kind="ExternalInput")
    crs_d = nc.dram_tensor("crsT", [128, 16], fp32, kind="ExternalInput")
    out_d = nc.dram_tensor("out", [3, 4 * ROWS_PER_CORE, OUT], fp32,
                           kind="ExternalOutput")

    with tile.TileContext(nc) as tc:
        with (
            tc.tile_pool(name="consts", bufs=1) as consts,
            tc.tile_pool(name="xin", bufs=3) as xin,
            tc.tile_pool(name="hbuf", bufs=3) as hbuf,
            tc.tile_pool(name="stage", bufs=4) as stage,
            tc.tile_pool(name="fpsum", bufs=2, space=bass.MemorySpace.PSUM) as fpsum,
            tc.tile_pool(name="ppsum", bufs=3, space=bass.MemorySpace.PSUM) as ppsum,
        ):
            w1_sb = consts.tile([128, 128], mm_dt)
            w2_sb = consts.tile([128, 6], mm_dt)
            crs_sb = consts.tile([128, 16], fp32)
            nc.sync.dma_start(w1_sb[:], w1_d[:])
            nc.sync.dma_start(w2_sb[:], w2_d[:])
            nc.sync.dma_start(crs_sb[:], crs_d[:])

            x_tiles = []
            f_tiles = []

            def load_x(t):
                xt = xin.tile([128, 4, W], mm_dt, tag="xt")
                # group A: rows 8t..8t+3 -> partitions 0..63 (64 channels)
                nc.sync.dma_start(xt[0:64, :, :], x_d[:, 8 * t:8 * t + 4, :])
                # group B: rows 8t+4..8t+7 -> partitions 64..127
                nc.sync.dma_start(xt[64:128, :, :], x_d[:, 8 * t + 4:8 * t + 8, :])
                x_tiles.append(xt)

            def feat_matmul(t):
                ft = fpsum.tile([128, 4, W], fp32, tag="ft")
                nc.tensor.matmul(ft[:, :, :], w1_sb[:], x_tiles[t][:, :, :],
                                 start=True, stop=True)
                f_tiles.append(ft)

            # 9 relu variants on ACT, 7 on DVE (balances 1.2 vs 0.96 GHz)
            ACT_V = {0, 2, 4, 6, 8, 10, 12, 14, 15}

            def tile_body(t):
                ft = f_tiles[t]
                for r in range(4):
                    # rhs tile ordered by output column: [part, i, l, s]
                    hr = hbuf.tile([128, 4, W, 4], mm_dt, tag="hr")
                    for s in range(4):
                        v = 4 * r + s
                        bias_ap = crs_sb[:, v:v + 1]
                        if v in ACT_V:
                            nc.scalar.activation(
                                hr[:, :, :, s], ft[:, :, :],
                                mybir.ActivationFunctionType.Relu,
                                bias=bias_ap)
                        else:
                            nc.vector.tensor_scalar(
                                hr[:, :, :, s], ft[:, :, :],
                                bias_ap, 0.0,
                                mybir.AluOpType.add, mybir.AluOpType.max)
                    if mm_dt == fp32:
                        # i-blocks at (partition 32*(i//2), slot i%2): matmul
                        # out base partition must be 0/32/64
                        pt = ppsum.tile([38, 2, OUT], fp32, tag="pt")
                        for i in range(4):
                            g, j = 32 * (i // 2), i % 2
                            nc.tensor.matmul(pt[g:g + 6, j, :], w2_sb[:],
                                             hr[:, i, :, :],
                                             start=True, stop=True)
                        st = stage.tile([38, 2, OUT], fp32, tag="st")
                        if r % 2 == 0:
                            nc.scalar.activation(
                                st[:, :, :], pt[:, :, :],
                                mybir.ActivationFunctionType.Copy)
                        else:
                            nc.vector.tensor_copy(st[:, :, :], pt[:, :, :])
                        # partitions g+3grp+c (grp: 0=A rows, 1=B rows+16)
                        for q in range(2):   # q = i//2 partition group
                            for grp in range(2):
                                row = 32 * t + 8 * q + 16 * grp + r
                                nc.sync.dma_start(
                                    out_d[:, row:row + 5:4, :],
                                    st[32 * q + 3 * grp:32 * q + 3 * grp + 3,
                                       :, :])
                    else:
                        # f32r: matmul dst base partition must be 0
                        for q in range(2):
                            pt = ppsum.tile([6, 2, OUT], fp32, tag="pt")
                            for j in range(2):
                                nc.tensor.matmul(pt[:, j, :], w2_sb[:],
                                                 hr[:, 2 * q + j, :, :],
                                                 start=True, stop=True)
                            st = stage.tile([6, 2, OUT], fp32, tag="st")
                            if (2 * r + q) % 2 == 0:
                                nc.scalar.activation(
                                    st[:, :, :], pt[:, :, :],
                                    mybir.ActivationFunctionType.Copy)
                            else:
                                nc.vector.tensor_copy(st[:, :, :],
                                                      pt[:, :, :])
                            for grp in range(2):
                                row = 32 * t + 8 * q + 16 * grp + r
                                nc.sync.dma_start(
                                    out_d[:, row:row + 5:4, :],
                                    st[3 * grp:3 * grp + 3, :, :])

            # software pipeline: F(t+1) issued before preds(t) so ACT/DVE
            # for tile t+1 overlap PE pred work of tile t
            load_x(0)
            feat_matmul(0)
            for t in range(T_TILES):
                if t + 1 < T_TILES:
                    load_x(t + 1)
                    feat_matmul(t + 1)
                tile_body(t)

    nc.compile()
    _CACHE["nc"] = nc
    return nc


def _prep_inputs(x, W1, b1, W2, b2):
    x = np.ascontiguousarray(np.asarray(x, dtype=np.float32))
    W1 = np.asarray(W1, dtype=np.float32)
    b1 = np.asarray(b1, dtype=np.float32)
    W2 = np.asarray(W2, dtype=np.float32)

    w1c = W1[:NF]                      # [64, 64]
    w1diag = np.zeros((128, 128), dtype=np.float32)
    w1diag[0:64, 0:64] = w1c
    w1diag[64:128, 64:128] = w1c

    w2diag = np.zeros((128, 6), dtype=np.float32)
    w2diag[0:64, 0:3] = W2
    w2diag[64:128, 3:6] = W2

    # c[r,s] = rel[r]*W1[64] + rel[s]*W1[65] + b1 -> [16, 64]
    crs = (REL[:, None, None] * W1[NF][None, None, :]
           + REL[None, :, None] * W1[NF + 1][None, None, :]
           + b1[None, None, :]).reshape(16, NF)
    crsT = np.ascontiguousarray(
        np.concatenate([crs.T, crs.T], axis=0))  # [128, 16]

    in_maps = []
    for c in range(N_CORES):
        b, half = c // 2, c % 2
        xs = np.ascontiguousarray(
            x[b, :, half * ROWS_PER_CORE:(half + 1) * ROWS_PER_CORE, :])
        in_maps.append({"x": xs, "w1diag": w1diag, "w2diag": w2diag,
                        "crsT": crsT})
    return in_maps


def _gather(results, b2):
    full = np.empty((B, 3, OUT, OUT), dtype=np.float32)
    for c in range(N_CORES):
        b, half = c // 2, c % 2
        full[b, :, half * (OUT // 2):(half + 1) * (OUT // 2), :] = \
            results[c]["out"]
    b2 = np.asarray(b2, dtype=np.float32)
    if np.any(b2):
        full += b2.reshape(1, 3, 1, 1)
    return full


def run(trace=False, **inputs):
    nc = _build_program()
    in_maps = _prep_inputs(inputs["x"], inputs["W1"], inputs["b1"],
                           inputs["W2"], inputs["b2"])
    res = run_bass_kernel_spmd(nc, in_maps, list(range(N_CORES)), trace=trace)
    return _gather(res.results, inputs["b2"]), res


def kernel(**inputs):
    out, _ = run(trace=False, **inputs)
    return out


# revision 13
# speedup vs baseline: 1.4132x; 1.3993x over previous
"""Trainium2 Bass kernel for nn_MLP_Interpolate.

Reference computation (out_size=512, H=W=128, so exact 4x nearest upsample):
  out[b, :, 4k+r, 4l+s] = relu(x[b,:,k,l] @ W1[:64] + c[r,s]) @ W2 + b2
  c[r,s] = rel_y(r)*W1[64] + rel_x(s)*W1[65] + b1,  rel(t) = (2t-3)/4

Strategy (8 cores, shard = (batch, H-half)):
  - F = W1c^T x computed on PE with a block-diagonal stationary so two
    64-channel pixel groups share one pass (128 partitions fully used).
  - 16 bias+relu variants split across ACT and DVE, written into an
    interleaved rhs tile ordered by *output* column (4l+s).
  - pred on PE with block-diag [128,6] W2 stationary -> PSUM rows are
    whole contiguous output rows, DMA'd straight to DRAM.
"""

import os

import numpy as np

import concourse.bass as bass
import concourse.bacc as bacc
import concourse.mybir as mybir
import concourse.tile as tile
from concourse.bass_utils import run_bass_kernel_spmd

# Problem constants (hardcoded per contract)
B, C, H, W = 4, 64, 128, 128
OUT = 512
NF = 64  # n_feat
N_CORES = 8
ROWS_PER_CORE = H // 2          # 64 input rows per core
T_TILES = ROWS_PER_CORE // 8    # 8 F-tiles, each covering 8 input rows
REL = np.array([-0.75, -0.25, 0.25, 0.75], dtype=np.float32)

_CACHE = {}


def _build_program():
    """Build + compile the SPMD Bass program once."""
    if "nc" in _CACHE:
        return _CACHE["nc"]

    fp32 = mybir.dt.float32
    # float32r: same bytes as fp32, PE streams 1 col/cycle vs 4 for fp32
    mm_dt = (mybir.dt.float32r if os.environ.get("MM_DTYPE") == "f32r"
             else fp32)
    nc = bacc.Bacc("TRN2", target_bir_lowering=False, debug=False,
                   num_devices=N_CORES)

    x_d = nc.dram_tensor("x", [C, ROWS_PER_CORE, W], mm_dt, kind="ExternalInput")
    w1_d = nc.dram_tensor("w1diag", [128, 128], mm_dt, kind="ExternalInput")
    w2_d = nc.dram_tensor("w2diag", [128, 6], mm_dt, kind="ExternalInput")
    crs_d = nc.dram_tensor("crsT", [128, 16], fp32, kind="ExternalInput")
    out_d = nc.dram_tensor("out", [3, 4 * ROWS_PER_CORE, OUT], fp32,
                           kind="ExternalOutput")

    with tile.TileContext(nc) as tc:
        with (
            tc.tile_pool(name="consts", bufs=1) as consts,
            tc.tile_pool(name="xin", bufs=3) as xin,
            tc.tile_pool(name="hbuf", bufs=3) as hbuf,
            tc.tile_pool(name="stage", bufs=4) as stage,
            tc.tile_pool(name="fpsum", bufs=2, space=bass.MemorySpace.PSUM) as fpsum,
            tc.tile_pool(name="ppsum", bufs=3, space=bass.MemorySpace.PSUM) as ppsum,
        ):
            w1_sb = consts.tile([128, 128], mm_dt)
            w2_sb = consts.tile([128, 6], mm_dt)
            crs_sb = consts.tile([128, 16], fp32)
            nc.sync.dma_start(w1_sb[:], w1_d[:])
            nc.sync.dma_start(w2_sb[:], w2_d[:])
            nc.sync.dma_start(crs_sb[:], crs_d[:])

            x_tiles = []
            f_tiles = []

            def load_x(t):
                xt = xin.tile([128, 4, W], mm_dt, tag="xt")
                # group A: rows 8t..8t+3 -> partitions 0..63 (64 channels)
                nc.sync.dma_start(xt[0:64, :, :], x_d[:, 8 * t:8 * t + 4, :])
                # group B: rows 8t+4..8t+7 -> partitions 64..127
                nc.sync.dma_start(xt[64:128, :, :], x_d[:, 8 * t + 4:8 * t + 8, :])
                x_tiles.append(xt)

            def feat_matmul(t):
                ft = fpsum.tile([128, 4, W], fp32, tag="ft")
                nc.tensor.matmul(ft[:, :, :], w1_sb[:], x_tiles[t][:, :, :],
                                 start=True, stop=True)
                f_tiles.append(ft)

            # 9 relu variants on ACT, 7 on DVE (balances 1.2 vs 0.96 GHz)
            ACT_V = {0, 2, 4, 6, 8, 10, 12, 14, 15}

            def tile_body(t):
                ft = f_tiles[t]
                for r in range(4):
                    # h tile [part, s, i, l]: relu writes contiguous 512-runs;
                    # the output-column interleave (4l+s) happens in the
                    # matmul rhs read AP instead (strided reads are free on
                    # PE, strided writes are ~2.7x on ACT/DVE)
                    hr = hbuf.tile([128, 4, 4, W], mm_dt, tag="hr")
                    for s in range(4):
                        v = 4 * r + s
                        bias_ap = crs_sb[:, v:v + 1]
                        if v in ACT_V:
                            nc.scalar.activation(
                                hr[:, s, :, :], ft[:, :, :],
                                mybir.ActivationFunctionType.Relu,
                                bias=bias_ap)
                        else:
                            nc.vector.tensor_scalar(
                                hr[:, s, :, :], ft[:, :, :],
                                bias_ap, 0.0,
                                mybir.AluOpType.add, mybir.AluOpType.max)
                    def mm_rhs(i):
                        # [l, s] with s innermost -> streamed col n = 4l+s
                        return hr[:, :, i, :].rearrange("p s l -> p l s")

                    if mm_dt == fp32:
                        # i-blocks at (partition 32*(i//2), slot i%2): matmul
                        # out base partition must be 0/32/64
                        pt = ppsum.tile([38, 2, OUT], fp32, tag="pt")
                        for i in range(4):
                            g, j = 32 * (i // 2), i % 2
                            nc.tensor.matmul(pt[g:g + 6, j, :], w2_sb[:],
                                             mm_rhs(i),
                                             start=True, stop=True)
                        st = stage.tile([38, 2, OUT], fp32, tag="st")
                        if r % 2 == 0:
                            nc.scalar.activation(
                                st[:, :, :], pt[:, :, :],
                                mybir.ActivationFunctionType.Copy)
                        else:
                            nc.vector.tensor_copy(st[:, :, :], pt[:, :, :])
                        # partitions g+3grp+c (grp: 0=A rows, 1=B rows+16)
                        for q in range(2):   # q = i//2 partition group
                            for grp in range(2):
                                row = 32 * t + 8 * q + 16 * grp + r
                                eng = nc.gpsimd if (q + grp) % 2 else nc.sync
                                eng.dma_start(
                                    out_d[:, row:row + 5:4, :],
                                    st[32 * q + 3 * grp:32 * q + 3 * grp + 3,
                                       :, :])
                    else:
                        # f32r: matmul dst base partition must be 0
                        st = stage.tile([6, 4, OUT], fp32, tag="st")
                        for q in range(2):
                            pt = ppsum.tile([6, 2, OUT], fp32, tag="pt")
                            for j in range(2):
                                nc.tensor.matmul(pt[:, j, :], w2_sb[:],
                                                 mm_rhs(2 * q + j),
                                                 start=True, stop=True)
                            if (2 * r + q) % 2 == 0:
                                nc.scalar.activation(
                                    st[:, 2 * q:2 * q + 2, :], pt[:, :, :],
                                    mybir.ActivationFunctionType.Copy)
                            else:
                                nc.vector.tensor_copy(
                                    st[:, 2 * q:2 * q + 2, :], pt[:, :, :])
                        for grp in range(2):
                            row = 32 * t + 16 * grp + r
                            eng = nc.gpsimd if grp else nc.sync
                            eng.dma_start(
                                out_d[:, row:row + 13:4, :],
                                st[3 * grp:3 * grp + 3, :, :])

            # software pipeline: F(t+1) issued before preds(t) so ACT/DVE
            # for tile t+1 overlap PE pred work of tile t
            load_x(0)
            feat_matmul(0)
            for t in range(T_TILES):
                if t + 1 < T_TILES:
                    load_x(t + 1)
                    feat_matmul(t + 1)
                tile_body(t)

    nc.compile()
    _CACHE["nc"] = nc
    return nc


def _prep_inputs(x, W1, b1, W2, b2):
    x = np.ascontiguousarray(np.asarray(x, dtype=np.float32))
    W1 = np.asarray(W1, dtype=np.float32)
    b1 = np.asarray(b1, dtype=np.float32)
    W2 = np.asarray(W2, dtype=np.float32)

    w1c = W1[:NF]                      # [64, 64]
    w1diag = np.zeros((128, 128), dtype=np.float32)
    w1diag[0:64, 0:64] = w1c
    w1diag[64:128, 64:128] = w1c

    w2diag = np.zeros((128, 6), dtype=np.float32)
    w2diag[0:64, 0:3] = W2
    w2diag[64:128, 3:6] = W2

    # c[r,s] = rel[r]*W1[64] + rel[s]*W1[65] + b1 -> [16, 64]
    crs = (REL[:, None, None] * W1[NF][None, None, :]
           + REL[None, :, None] * W1[NF + 1][None, None, :]
           + b1[None, None, :]).reshape(16, NF)
    crsT = np.ascontiguousarray(
        np.concatenate([crs.T, crs.T], axis=0))  # [128, 16]

    in_maps = []
    for c in range(N_CORES):
        b, half = c // 2, c % 2
        xs = np.ascontiguousarray(
            x[b, :, half * ROWS_PER_CORE:(half + 1) * ROWS_PER_CORE, :])
        in_maps.append({"x": xs, "w1diag": w1diag, "w2diag": w2diag,
                        "crsT": crsT})
    return in_maps


def _gather(results, b2):
    full = np.empty((B, 3, OUT, OUT), dtype=np.float32)
    for c in range(N_CORES):
        b, half = c // 2, c % 2
        full[b, :, half * (OUT // 2):(half + 1) * (OUT // 2), :] = \
            results[c]["out"]
    b2 = np.asarray(b2, dtype=np.float32)
    if np.any(b2):
        full += b2.reshape(1, 3, 1, 1)
    return full


def run(trace=False, **inputs):
    nc = _build_program()
    in_maps = _prep_inputs(inputs["x"], inputs["W1"], inputs["b1"],
                           inputs["W2"], inputs["b2"])
    res = run_bass_kernel_spmd(nc, in_maps, list(range(N_CORES)), trace=trace)
    return _gather(res.results, inputs["b2"]), res


def kernel(**inputs):
    out, _ = run(trace=False, **inputs)
    return out


# revision 14
# speedup vs baseline: 1.4584x; 1.0320x over previous
"""Trainium2 Bass kernel for nn_MLP_Interpolate.

Reference computation (out_size=512, H=W=128, so exact 4x nearest upsample):
  out[b, :, 4k+r, 4l+s] = relu(x[b,:,k,l] @ W1[:64] + c[r,s]) @ W2 + b2
  c[r,s] = rel_y(r)*W1[64] + rel_x(s)*W1[65] + b1,  rel(t) = (2t-3)/4

Strategy (8 cores, shard = (batch, H-half)):
  - F = W1c^T x computed on PE with a block-diagonal stationary so two
    64-channel pixel groups share one pass (128 partitions fully used).
  - 16 bias+relu variants split across ACT and DVE, written into an
    interleaved rhs tile ordered by *output* column (4l+s).
  - pred on PE with block-diag [128,6] W2 stationary -> PSUM rows are
    whole contiguous output rows, DMA'd straight to DRAM.
"""

import os

import numpy as np

import concourse.bass as bass
import concourse.bacc as bacc
import concourse.mybir as mybir
import concourse.tile as tile
from concourse.bass_utils import run_bass_kernel_spmd

# Problem constants (hardcoded per contract)
B, C, H, W = 4, 64, 128, 128
OUT = 512
NF = 64  # n_feat
N_CORES = 8
ROWS_PER_CORE = H // 2          # 64 input rows per core
T_TILES = ROWS_PER_CORE // 8    # 8 F-tiles, each covering 8 input rows
REL = np.array([-0.75, -0.25, 0.25, 0.75], dtype=np.float32)

_CACHE = {}


def _build_program():
    """Build + compile the SPMD Bass program once."""
    if "nc" in _CACHE:
        return _CACHE["nc"]

    fp32 = mybir.dt.float32
    # float32r: same bytes as fp32, PE streams 1 col/cycle vs 4 for fp32
    mm_dt = (mybir.dt.float32r if os.environ.get("MM_DTYPE") == "f32r"
             else fp32)
    nc = bacc.Bacc("TRN2", target_bir_lowering=False, debug=False,
                   num_devices=N_CORES)

    x_d = nc.dram_tensor("x", [C, ROWS_PER_CORE, W], mm_dt, kind="ExternalInput")
    w1_d = nc.dram_tensor("w1diag", [128, 128], mm_dt, kind="ExternalInput")
    w2_d = nc.dram_tensor("w2diag", [128, 6], mm_dt, kind="ExternalInput")
    crs_d = nc.dram_tensor("crsT", [128, 16], fp32, kind="ExternalInput")
    out_d = nc.dram_tensor("out", [3, 4 * ROWS_PER_CORE, OUT], fp32,
                           kind="ExternalOutput")

    NT = ROWS_PER_CORE // 16  # 4 F-tiles, each 16 input rows (8 per group)

    with tile.TileContext(nc) as tc:
        with (
            tc.tile_pool(name="consts", bufs=1) as consts,
            tc.tile_pool(name="xin", bufs=2) as xin,
            tc.tile_pool(name="hbuf", bufs=2) as hbuf,
            tc.tile_pool(name="stage", bufs=6) as stage,
            tc.tile_pool(name="fpsum", bufs=2, space=bass.MemorySpace.PSUM) as fpsum,
            tc.tile_pool(name="ppsum", bufs=2, space=bass.MemorySpace.PSUM) as ppsum,
        ):
            w1_sb = consts.tile([128, 128], mm_dt)
            w2_sb = consts.tile([128, 6], mm_dt)
            crs_sb = consts.tile([128, 16], fp32)
            nc.sync.dma_start(w1_sb[:], w1_d[:])
            nc.sync.dma_start(w2_sb[:], w2_d[:])
            nc.sync.dma_start(crs_sb[:], crs_d[:])

            x_tiles = []
            f_tiles = []

            def load_x(t):
                xt = xin.tile([128, 8, W], mm_dt, tag="xt")
                # group A: rows 16t..16t+8 -> partitions 0..63 (64 channels)
                nc.sync.dma_start(xt[0:64, :, :], x_d[:, 16 * t:16 * t + 8, :])
                # group B: rows 16t+8..16t+16 -> partitions 64..127
                nc.gpsimd.dma_start(xt[64:128, :, :],
                                    x_d[:, 16 * t + 8:16 * t + 16, :])
                x_tiles.append(xt)

            def feat_matmul(t):
                ft = fpsum.tile([128, 8, W], fp32, tag="ft")
                for half in range(2):
                    nc.tensor.matmul(ft[:, 4 * half:4 * half + 4, :],
                                     w1_sb[:],
                                     x_tiles[t][:, 4 * half:4 * half + 4, :],
                                     start=True, stop=True)
                f_tiles.append(ft)

            # 10 relu variants on ACT, 6 on DVE; copies 3 ACT / 5 DVE
            ACT_V = {0, 2, 4, 6, 8, 10, 12, 14, 15, 13}

            def tile_body(t):
                ft = f_tiles[t]
                for r in range(4):
                    # h tile [part, s, i, l]: relu writes contiguous runs;
                    # the output-column interleave (4l+s) happens in the
                    # matmul rhs read AP instead (strided reads are free on
                    # PE, strided writes are ~2.7x on ACT/DVE)
                    hr = hbuf.tile([128, 4, 8, W], mm_dt, tag="hr")
                    for s in range(4):
                        v = 4 * r + s
                        bias_ap = crs_sb[:, v:v + 1]
                        if v in ACT_V:
                            nc.scalar.activation(
                                hr[:, s, :, :], ft[:, :, :],
                                mybir.ActivationFunctionType.Relu,
                                bias=bias_ap)
                        else:
                            nc.vector.tensor_scalar(
                                hr[:, s, :, :], ft[:, :, :],
                                bias_ap, 0.0,
                                mybir.AluOpType.add, mybir.AluOpType.max)

                    def mm_rhs(i):
                        # [l, s] with s innermost -> streamed col n = 4l+s
                        return hr[:, :, i, :].rearrange("p s l -> p l s")

                    copy_idx = 0
                    for ihalf in range(2):
                        if mm_dt == fp32:
                            # i-quad at (partition 32*(ii//2), slot ii%2)
                            pt = ppsum.tile([38, 2, OUT], fp32, tag="pt")
                            for ii in range(4):
                                g, j = 32 * (ii // 2), ii % 2
                                nc.tensor.matmul(pt[g:g + 6, j, :], w2_sb[:],
                                                 mm_rhs(4 * ihalf + ii),
                                                 start=True, stop=True)
                            st = stage.tile([38, 2, OUT], fp32, tag="st")
                            if (r + ihalf) % 2 == 0:
                                nc.scalar.activation(
                                    st[:, :, :], pt[:, :, :],
                                    mybir.ActivationFunctionType.Copy)
                            else:
                                nc.vector.tensor_copy(st[:, :, :],
                                                      pt[:, :, :])
                            for q in range(2):
                                for grp in range(2):
                                    row = (64 * t + 16 * ihalf + 8 * q
                                           + 32 * grp + r)
                                    eng = (nc.gpsimd if (q + grp) % 2
                                           else nc.sync)
                                    eng.dma_start(
                                        out_d[:, row:row + 5:4, :],
                                        st[32 * q + 3 * grp:
                                           32 * q + 3 * grp + 3, :, :])
                        else:
                            # f32r: matmul dst base partition must be 0
                            st = stage.tile([6, 4, OUT], fp32, tag="st")
                            for jj in range(2):
                                pt = ppsum.tile([6, 2, OUT], fp32, tag="pt")
                                for j in range(2):
                                    i = 4 * ihalf + 2 * jj + j
                                    nc.tensor.matmul(pt[:, j, :], w2_sb[:],
                                                     mm_rhs(i),
                                                     start=True, stop=True)
                                # copies: 3 on ACT, 5 on DVE per r-loop pair
                                if copy_idx in (0, 3):
                                    nc.scalar.activation(
                                        st[:, 2 * jj:2 * jj + 2, :],
                                        pt[:, :, :],
                                        mybir.ActivationFunctionType.Copy)
                                else:
                                    nc.vector.tensor_copy(
                                        st[:, 2 * jj:2 * jj + 2, :],
                                        pt[:, :, :])
                                copy_idx += 1
                            for grp in range(2):
                                row = 64 * t + 16 * ihalf + 32 * grp + r
                                eng = nc.gpsimd if grp else nc.sync
                                eng.dma_start(
                                    out_d[:, row:row + 13:4, :],
                                    st[3 * grp:3 * grp + 3, :, :])

            # software pipeline: F(t+1) issued before preds(t) so ACT/DVE
            # for tile t+1 overlap PE pred work of tile t
            load_x(0)
            feat_matmul(0)
            for t in range(NT):
                if t + 1 < NT:
                    load_x(t + 1)
                    feat_matmul(t + 1)
                tile_body(t)

    nc.compile()
    _CACHE["nc"] = nc
    return nc


def _prep_inputs(x, W1, b1, W2, b2):
    x = np.ascontiguousarray(np.asarray(x, dtype=np.float32))
    W1 = np.asarray(W1, dtype=np.float32)
    b1 = np.asarray(b1, dtype=np.float32)
    W2 = np.asarray(W2, dtype=np.float32)

    w1c = W1[:NF]                      # [64, 64]
    w1diag = np.zeros((128, 128), dtype=np.float32)
    w1diag[0:64, 0:64] = w1c
    w1diag[64:128, 64:128] = w1c

    w2diag = np.zeros((128, 6), dtype=np.float32)
    w2diag[0:64, 0:3] = W2
    w2diag[64:128, 3:6] = W2

    # c[r,s] = rel[r]*W1[64] + rel[s]*W1[65] + b1 -> [16, 64]
    crs = (REL[:, None, None] * W1[NF][None, None, :]
           + REL[None, :, None] * W1[NF + 1][None, None, :]
           + b1[None, None, :]).reshape(16, NF)
    crsT = np.ascontiguousarray(
        np.concatenate([crs.T, crs.T], axis=0))  # [128, 16]

    in_maps = []
    for c in range(N_CORES):
        b, half = c // 2, c % 2
        xs = np.ascontiguousarray(
            x[b, :, half * ROWS_PER_CORE:(half + 1) * ROWS_PER_CORE, :])
        in_maps.append({"x": xs, "w1diag": w1diag, "w2diag": w2diag,
                        "crsT": crsT})
    return in_maps


def _gather(results, b2):
    full = np.empty((B, 3, OUT, OUT), dtype=np.float32)
    for c in range(N_CORES):
        b, half = c // 2, c % 2
        full[b, :, half * (OUT // 2):(half + 1) * (OUT // 2), :] = \
            results[c]["out"]
    b2 = np.asarray(b2, dtype=np.float32)
    if np.any(b2):
        full += b2.reshape(1, 3, 1, 1)
    return full


def run(trace=False, **inputs):
    nc = _build_program()
    in_maps = _prep_inputs(inputs["x"], inputs["W1"], inputs["b1"],
                           inputs["W2"], inputs["b2"])
    res = run_bass_kernel_spmd(nc, in_maps, list(range(N_CORES)), trace=trace)
    return _gather(res.results, inputs["b2"]), res


def kernel(**inputs):
    out, _ = run(trace=False, **inputs)
    return out
